# revision 48
# baseline (speedup 1.0000x reference)
"""Incremental MADE autoregressive sampler on 8 TRN2 NeuronCores.

v2: ALL layer accumulators are persistent PSUM banks updated incrementally.

With hidden units degree-sorted, activations are append-only across the 32
autoregressive steps: once x_0..x_g are set, every hidden unit of degree <= g
is final. Exploit this at every layer:

- pre1 (L1 pre-activations): ping-pong PSUM banks per 128-block; one K=1
  rank-1 update per step (new coordinate), plus a single K=33 catch-up matmul
  from xT (with a ones-row folding in the bias) when a block first becomes
  the active cover.
- S2/S3 (L2/L3 pre-activations): ping-pong PSUM banks per cover block. Each
  step adds ONLY the newly finalized ~33-unit degree group (K~33 matmul).
  When a block is about to become cover, a catch-up chain (bias + full
  finalized chunks) accumulates the older prefix once. No per-step prefix
  recompute -> Tensor queue no longer head-of-line-blocks the serial chain.
- theta [64, B]: single persistent PSUM accumulator; per-step "touch" adds
  the new group's contribution to all output rows (K~33, M=64). The tail
  reads rows idx (mu) and 32+idx (log_std) straight out of PSUM.
- Tail: es=exp(theta_ls) (ACT) -> t2=es*z -> x_idx=t2+theta_mu written
  DIRECTLY into the xT row in SBUF (no DMA scatter; k1/catch-up read xT).

Batch: data-parallel over 8 cores (512 rows/core); each core runs two
256-wide column chains, phase-interleaved with a skew so the two serial
dependency chains overlap on different engines. Relus/tails are spread
across ACT/DVE/Pool(gpsimd) so no single elementwise engine saturates.
"""

import os
import sys
import numpy as np

for _p in ("/opt/trn_rl_repo", "/opt/pypackages"):
    if _p not in sys.path:
        sys.path.insert(0, _p)

import concourse.bass as bass
import concourse.tile as tile
from concourse import bacc
from concourse import mybir
from concourse.bass_utils import run_bass_kernel_spmd

D, H, L, B = 32, 1024, 2, 4096
NCORES = 8
BC = B // NCORES          # 512 batch rows per core
P = 128                   # partitions
NB = H // P               # 8 hidden blocks
F32 = mybir.dt.float32
MMDT = mybir.dt.bfloat16

STOP = int(os.environ.get("MADE_STOP", "32"))
NCH = int(os.environ.get("MADE_CHAINS", "2"))
SKEW = int(os.environ.get("MADE_SKEW", "7"))


def _schedule():
    """Static per-step schedule from the degree structure."""
    d_hid = np.arange(H) % (D - 1)
    perm = np.argsort(d_hid, kind="stable")
    ds = d_hid[perm]
    glo = [int(np.sum(ds < g)) for g in range(D - 1)]
    ghi = [int(np.sum(ds <= g)) for g in range(D - 1)]
    cov = {g: list(range(glo[g] // P, (ghi[g] - 1) // P + 1))
           for g in range(D - 1)}
    # src_slices(g): (block c, row lo, row hi) covering units [glo, ghi)
    srcs = {}
    for g in range(D - 1):
        sl = []
        for c in cov[g]:
            lo = max(glo[g], c * P) - c * P
            hi = min(ghi[g], (c + 1) * P) - c * P
            sl.append((c, lo, hi))
        srcs[g] = sl
    # first step idx at which block Bb is in cover
    first = {}
    for g in range(D - 1):
        for Bb in cov[g]:
            first.setdefault(Bb, g + 1)
    return perm, ds, glo, ghi, cov, srcs, first


def _host_prep(W0, b0, Wh, bh, Wout, bout):
    perm, ds, glo, ghi, cov, srcs, first = _schedule()
    d_in = np.arange(D)
    d_out = np.arange(D) - 1
    m0 = (ds[:, None] >= d_in[None, :]).astype(np.float32)        # [H, D]
    mh = (ds[:, None] >= ds[None, :]).astype(np.float32)          # [H, H]
    mo = (d_out[:, None] >= ds[None, :]).astype(np.float32)       # [D, H]
    mo = np.concatenate([mo, mo], axis=0)                         # [2D, H]

    W0p = m0 * W0[perm, :]                    # [H, D] (out, in)
    Wh0p = mh * Wh[0][perm][:, perm]          # [H, H] (out, in)
    Wh1p = mh * Wh[1][perm][:, perm]
    Wop = mo * Wout[:, perm]                  # [2D, H]

    w0b = np.concatenate([W0p.T, b0[perm][None, :]], axis=0)      # [D+1, H]
    wh0T = Wh0p.T.reshape(NB, P, H).copy()                        # [c][128, H]
    wh1T = Wh1p.T.reshape(NB, P, H).copy()

    # k1 packed columns: per (idx, Bb in cov(idx-1)): W0p[block Bb, idx-1]
    k1_off, k1_list = {}, []
    for idx in range(1, D):
        for Bb in cov[idx - 1]:
            k1_off[(idx, Bb)] = len(k1_list)
            k1_list.append(W0p[Bb * P:(Bb + 1) * P, idx - 1])
    w0k1 = np.concatenate(k1_list).reshape(1, -1)                 # [1, n*128]

    # slice maps for the device-built masked lhsT tensors
    g2_off, n_g = {}, 0
    for idx in range(1, D):
        g = idx - 1
        for Bb in cov[g]:
            for (c, lo, hi) in srcs[g]:
                g2_off[(idx, Bb, c)] = n_g
                n_g += 1
    to_off = {}
    for idx in range(1, D):
        for (c, lo, hi) in srcs[idx - 1]:
            to_off[(idx, c)] = len(to_off)
    n_to = len(to_off)

    # distinct group-row masks: per (g, c): rows [lo, hi) of block c
    m_off, m_cols = {}, []
    for g in range(D - 1):
        for (c, lo, hi) in srcs[g]:
            col = np.zeros(P, dtype=np.float32)
            col[lo:hi] = 1.0
            m_off[(g, c)] = len(m_cols)
            m_cols.append(col)
    mcol = np.stack(m_cols, axis=1)                               # [128, nm]

    # S2/S3 catch-up schedule (mirrors _build's hc_sched pair usage):
    # used (c src, Bb dst) weight blocks = catch-up pairs + group pairs
    pair_use = {}

    def use(c, Bb, step):
        k = (c, Bb)
        pair_use[k] = min(pair_use.get(k, 99), step)

    for Bb in range(1, NB):
        f = first[Bb]
        U = ghi[f - 2]
        cfull, rem = U // P, U % P
        for c in range(cfull + (1 if rem else 0)):
            use(c, Bb, max(1, f - 3))
    for idx in range(1, D):
        g = idx - 1
        for Bb in cov[g]:
            for (c, lo, hi) in srcs[g]:
                use(c, Bb, max(1, idx - 3))

    pairs = sorted(pair_use, key=lambda k: (pair_use[k], k))
    ub = {k: j for j, k in enumerate(pairs)}
    whu2 = np.concatenate([wh0T[c][:, Bb * P:(Bb + 1) * P]
                           for (c, Bb) in pairs], axis=1)         # [128, n*128]
    whu3 = np.concatenate([wh1T[c][:, Bb * P:(Bb + 1) * P]
                           for (c, Bb) in pairs], axis=1)

    # full output weights (small): wou[:, 64c:64(c+1)] = Wop[:, block c].T
    wou = np.concatenate([Wop[:, c * P:(c + 1) * P].T
                          for c in range(NB)], axis=1)            # [128, NB*2D]

    # one-hot theta_sb selectors, M=33 per idx: col 0 = e_{D+idx}, col 32 = e_idx
    M3 = D + 1
    ohx = np.zeros((2 * D, D * M3), dtype=np.float32)
    for idx in range(D):
        ohx[D + idx, M3 * idx] = 1.0
        ohx[idx, M3 * idx + D] = 1.0

    return dict(w0b=w0b, w0k1=w0k1, whu2=whu2, whu3=whu3, wou=wou,
                mcol=mcol, ohx=ohx,
                k1_off=k1_off, g2_off=g2_off, to_off=to_off, m_off=m_off,
                ub=ub, n_pairs=len(pairs), pair_use=pair_use,
                n_k1=len(k1_list), n_g2=n_g, n_to=n_to, n_m=len(m_cols),
                bh0r=bh[0][perm][None, :], bh1r=bh[1][perm][None, :],
                boutr=bout[None, :],
                glo=glo, ghi=ghi, cov=cov, srcs=srcs, first=first, ds=ds)


def _build(prep):
    nc = bacc.Bacc("TRN2", target_bir_lowering=False, debug=False,
                   num_devices=NCORES)

    def din(name, shape, dt=MMDT):
        return nc.dram_tensor(name, list(shape), dt, kind="ExternalInput").ap()

    NPAIR = prep["n_pairs"]
    d_w0b = din("w0b", (D + 1, H))
    d_whu2 = din("whu2", (P, NPAIR * P))
    d_whu3 = din("whu3", (P, NPAIR * P))
    d_wou = din("wou", (P, NB * 2 * D))
    d_mcol = din("mcol", (P, prep["n_m"]), F32)
    d_w0k1 = din("w0k1", (1, prep["n_k1"] * P))
    d_ohx = din("ohx", (2 * D, D * (D + 1)))
    d_bh0 = din("bh0r", (1, H))
    d_bh1 = din("bh1r", (1, H))
    d_bo = din("boutr", (1, 2 * D))
    d_z = din("zb", (D, BC), F32)
    d_out = nc.dram_tensor("out", [D, BC], F32, kind="ExternalOutput").ap()

    cov, srcs, first = prep["cov"], prep["srcs"], prep["first"]
    ghi, dsl = prep["ghi"], prep["ds"]
    # pre1 catch-up for block Bb is emitted during step first[Bb]-1
    catch_at = {}
    for Bb, f in first.items():
        if Bb >= 1:
            catch_at.setdefault(f - 1, []).append(Bb)

    # S2/S3 catch-up terms, spread over steps f-3..f-1 by data availability
    # (chunk c of the prefix is final once its last unit's group is done).
    # Terms of one (lyr, Bb) accumulation group stay in order; start/stop
    # flags mark the PSUM group boundaries.
    hc_sched = {}
    for lyr in (2, 3):
        for Bb in range(1, NB):
            f = first[Bb]
            U = ghi[f - 2]
            cfull, rem = U // P, U % P
            terms = [("bias", Bb)]
            terms += [("chunk", Bb, c) for c in range(cfull)]
            if rem:
                terms.append(("part", Bb, cfull, rem))
            n = len(terms)
            for j, t in enumerate(terms):
                if t[0] == "bias":
                    e = f - 3
                elif t[0] == "chunk":
                    e = max(f - 3, int(dsl[(t[2] + 1) * P - 1]) + 1)
                else:
                    e = f - 1
                e = min(max(e, 1), f - 1)
                hc_sched.setdefault((lyr, e), []).append((t, j == 0, j == n - 1))

    from contextlib import ExitStack
    with tile.TileContext(nc) as tc, ExitStack() as ctx:
        cp = ctx.enter_context(tc.tile_pool(name="const", bufs=1))
        pp = ctx.enter_context(tc.tile_pool(name="pers", bufs=1, space="PSUM"))

        w0b = cp.tile([D + 1, H], MMDT, tag="w0b")
        whu2 = cp.tile([P, NPAIR * P], MMDT, tag="whu2")
        whu3 = cp.tile([P, NPAIR * P], MMDT, tag="whu3")
        wou = cp.tile([P, NB * 2 * D], MMDT, tag="wou")
        mcol = cp.tile([P, prep["n_m"]], F32, tag="mcol")
        w0k1 = cp.tile([1, prep["n_k1"] * P], MMDT, tag="w0k1")
        # device-built masked lhsT slices (GPSIMD fills these from whu/wou)
        wgs2 = cp.tile([P, prep["n_g2"] * P], MMDT, tag="wgs2")
        wgs3 = cp.tile([P, prep["n_g2"] * P], MMDT, tag="wgs3")
        wtoS = cp.tile([P, prep["n_to"] * 2 * D], MMDT, tag="wtoS")
        wpmS = cp.tile([P, prep["n_to"] * (D + 1)], MMDT, tag="wpmS")
        bh0r = cp.tile([1, H], MMDT, tag="bh0r")
        bh1r = cp.tile([1, H], MMDT, tag="bh1r")
        bor = cp.tile([1, 2 * D], MMDT, tag="bor")
        ohx = cp.tile([2 * D, D * (D + 1)], MMDT, tag="ohx")
        theta_sb = cp.tile([2 * D, BC], MMDT, tag="theta_sb")
        ones = cp.tile([1, BC], MMDT, tag="ones")
        xT = cp.tile([D + 1, BC], MMDT, tag="xT")
        a1 = [cp.tile([P, BC], MMDT, tag=f"a1_{r}", name=f"a1_{r}") for r in range(NB)]
        a2 = [cp.tile([P, BC], MMDT, tag=f"a2_{r}", name=f"a2_{r}") for r in range(NB)]
        a3 = [cp.tile([P, BC], MMDT, tag=f"a3_{r}", name=f"a3_{r}") for r in range(NB)]
        es = cp.tile([1, BC], F32, tag="es")
        t2 = cp.tile([1, BC], F32, tag="t2")
        xi = [cp.tile([1, BC], MMDT, tag=f"xi{p}", name=f"xi{p}")
              for p in range(2)]
        outf = cp.tile([D, BC], F32, tag="outf")

        # persistent PSUM: pre1/S2/S3 ping-pong banks + theta accumulator +
        # extract tile (ls at row 0, mu at row 32: both %32-aligned)
        pre1 = [pp.tile([P, BC], F32, tag=f"pre1_{s}", name=f"pre1_{s}")
                for s in range(2)]
        s2 = [pp.tile([P, BC], F32, tag=f"s2_{s}", name=f"s2_{s}")
              for s in range(2)]
        s3 = [pp.tile([P, BC], F32, tag=f"s3_{s}", name=f"s3_{s}")
              for s in range(2)]
        theta = pp.tile([2 * D, BC], F32, tag="theta")
        ex = pp.tile([D + 1, BC], F32, tag="ex")

        # input DMAs, ordered by first use; whu columns are first-use ordered
        # so quarter k arrives before the steps that need it.
        nc.sync.dma_start(bor[:], d_bo)
        nc.sync.dma_start(ohx[:], d_ohx)
        nc.sync.dma_start(w0b[:], d_w0b)
        nc.sync.dma_start(w0k1[:], d_w0k1)
        nc.sync.dma_start(mcol[:], d_mcol)
        nc.sync.dma_start(wou[:], d_wou)
        nc.sync.dma_start(bh0r[:], d_bh0)
        nc.sync.dma_start(bh1r[:], d_bh1)
        bnd = [0] + [NPAIR * P * k // 4 for k in (1, 2, 3)] + [NPAIR * P]
        for a, b in zip(bnd, bnd[1:]):
            nc.sync.dma_start(whu2[:, a:b], d_whu2[:, a:b])
            nc.sync.dma_start(whu3[:, a:b], d_whu3[:, a:b])

        zrow = {}

        def fetch_z(i):
            if i < STOP and i not in zrow:
                zr_t = cp.tile([1, BC], F32, tag="zrow", bufs=4, name=f"zr{i}")
                zrow[i] = zr_t
                nc.sync.dma_start(zr_t[:], d_z[i:i + 1, :])

        for i in range(3):
            fetch_z(i)

        nc.vector.memset(xT[:], 0.0)
        nc.vector.memset(xT[D:D + 1, :], 1.0)
        nc.vector.memset(ones[:], 1.0)

        # theta := bout broadcast (rank-1); block-0 accumulators: bias+coords
        nc.tensor.matmul(theta[:, :], bor[0:1, :], ones[0:1, :],
                         start=True, stop=True, skip_group_check=True)
        nc.vector.tensor_scalar_add(theta_sb[:, :], theta[:, :], 0.0)
        nc.tensor.matmul(pre1[0], w0b[:, 0:P], xT[:, :],
                         start=True, stop=True, skip_group_check=True)
        nc.tensor.matmul(s2[0], bh0r[0:1, 0:P], ones[0:1, :],
                         start=True, stop=True, skip_group_check=True)
        nc.tensor.matmul(s3[0], bh1r[0:1, 0:P], ones[0:1, :],
                         start=True, stop=True, skip_group_check=True)

        CWX = BC // NCH
        chs = [(ch, slice(ch * CWX, (ch + 1) * CWX)) for ch in range(NCH)]

        # engine spread: (chain, layer 1/2/3) -> relu engine.
        # GPSIMD/Pool cannot access PSUM, so PSUM-reading ops (relu/exp/add)
        # go to ACT+DVE; Pool gets the SBUF-only tail multiply.
        RELU_ENG = {(0, 1): "act", (0, 2): "dve", (0, 3): "act",
                    (1, 1): "dve", (1, 2): "act", (1, 3): "dve"}

        def relu_op(eng, out_ap, in_ap):
            if eng == "act":
                nc.scalar.activation(out_ap, in_ap,
                                     mybir.ActivationFunctionType.Relu)
            elif eng == "dve":
                nc.vector.tensor_scalar_max(out_ap, in_ap, 0.0)
            else:
                nc.gpsimd.tensor_scalar_max(out_ap, in_ap, 0.0)



        streams = [[] for _ in range(NCH)]

        def ph(ch, fn):
            streams[ch].append(fn)

        def mk(fn, *args):
            return lambda a=args: fn(*a)

        k1_off = prep["k1_off"]
        g2_off, to_off = prep["g2_off"], prep["to_off"]
        m_off, ub = prep["m_off"], prep["ub"]
        M3 = D + 1

        # ---- device-side masked-slice builds (GPSIMD, off critical path) ----
        nc.gpsimd.memset(wpmS[:], 0.0)

        def build_grp_slice(idx, Bb, c):
            g = idx - 1
            off = g2_off[(idx, Bb, c)]
            m = m_off[(g, c)]
            src = ub[(c, Bb)]
            for whu, wgs in ((whu2, wgs2), (whu3, wgs3)):
                nc.gpsimd.tensor_scalar_mul(
                    wgs[:, off * P:(off + 1) * P],
                    whu[:, src * P:(src + 1) * P], mcol[:, m:m + 1])

        def build_out_slice(idx, c):
            g = idx - 1
            off = to_off[(idx, c)]
            m = m_off[(g, c)]
            nc.gpsimd.tensor_scalar_mul(
                wtoS[:, off * 2 * D:(off + 1) * 2 * D],
                wou[:, c * 2 * D:(c + 1) * 2 * D], mcol[:, m:m + 1])
            for k, oc in ((0, D + idx), (D, idx)):
                nc.gpsimd.tensor_scalar_mul(
                    wpmS[:, off * M3 + k:off * M3 + k + 1],
                    wou[:, c * 2 * D + oc:c * 2 * D + oc + 1],
                    mcol[:, m:m + 1])

        bsched = {}
        for idx in range(1, STOP):
            g = idx - 1
            at = max(0, idx - 3)
            for Bb in cov[g]:
                for (c, lo, hi) in srcs[g]:
                    bsched.setdefault(at, []).append(
                        lambda a=(idx, Bb, c): build_grp_slice(*a))
            for (c, lo, hi) in srcs[g]:
                bsched.setdefault(at, []).append(
                    lambda a=(idx, c): build_out_slice(*a))

        for fn in bsched.get(0, []):
            fn()

        def emit_catchup(ch, hs, idx):
            for Bb in catch_at.get(idx, []):
                nc.tensor.matmul(pre1[Bb % 2][:, hs],
                                 w0b[:, Bb * P:(Bb + 1) * P], xT[:, hs],
                                 start=True, stop=True, skip_group_check=True)

        def emit_k1(ch, hs, idx):
            g = idx - 1
            for Bb in cov[g]:
                off = k1_off[(idx, Bb)]
                nc.tensor.matmul(pre1[Bb % 2][:, hs],
                                 w0k1[0:1, off * P:(off + 1) * P],
                                 xi[(idx - 1) % 2][0:1, hs],
                                 start=False, stop=True, skip_group_check=True)

        def emit_relu(ch, hs, idx, lyr):
            g = idx - 1
            src, dst = {1: (pre1, a1), 2: (s2, a2), 3: (s3, a3)}[lyr]
            for Bb in cov[g]:
                relu_op(RELU_ENG[(ch % 2, lyr)], dst[Bb][:, hs],
                        src[Bb % 2][:, hs])

        def emit_grp(ch, hs, idx, lyr):
            """Add the newly final group g to cover-block accumulators."""
            g = idx - 1
            wg, sb, a_in = {2: (wgs2, s2, a1), 3: (wgs3, s3, a2)}[lyr]
            for Bb in cov[g]:
                for (c, lo, hi) in srcs[g]:
                    off = g2_off[(idx, Bb, c)]
                    nc.tensor.matmul(sb[Bb % 2][:, hs],
                                     wg[:, off * P:(off + 1) * P],
                                     a_in[c][:, hs],
                                     start=False, stop=True,
                                     skip_group_check=True)

        def emit_hid_catchup(ch, hs, idx, lyr):
            """Accumulate bias + finalized prefix for soon-to-be-cover blocks."""
            whu, sb, a_in, bias = {2: (whu2, s2, a1, bh0r),
                                   3: (whu3, s3, a2, bh1r)}[lyr]
            for (t, is_start, is_stop) in hc_sched.get((lyr, idx), []):
                Bb = t[1]
                dst = sb[Bb % 2][:, hs]
                if t[0] == "bias":
                    lh, rh = bias[0:1, Bb * P:(Bb + 1) * P], ones[0:1, hs]
                elif t[0] == "chunk":
                    c = t[2]
                    j = ub[(c, Bb)]
                    lh, rh = whu[:, j * P:(j + 1) * P], a_in[c][:, hs]
                else:
                    c, rem = t[2], t[3]
                    j = ub[(c, Bb)]
                    lh, rh = (whu[0:rem, j * P:(j + 1) * P],
                              a_in[c][0:rem, hs])
                nc.tensor.matmul(dst, lh, rh, start=is_start, stop=is_stop,
                                 skip_group_check=True)

        def emit_extract(ch, hs, idx):
            # One M=33 chain: ls_idx -> ex row 0, mu_idx -> ex row 32 (both
            # %32-aligned). Term 0 selects the theta_sb state rows (groups
            # <= idx-2); wpm terms add the current group's correction.
            cor = srcs[idx - 1] if idx >= 1 else []
            seq = [(ohx[:, M3 * idx:M3 * (idx + 1)], theta_sb[:, hs])]
            for (c, lo, hi) in cor:
                off = to_off[(idx, c)]
                seq.append((wpmS[:, M3 * off:M3 * (off + 1)], a3[c][:, hs]))
            for j, (lh, rh) in enumerate(seq):
                nc.tensor.matmul(ex[:, hs], lh, rh,
                                 start=(j == 0), stop=(j == len(seq) - 1),
                                 skip_group_check=True)

        def emit_touch(ch, hs, idx):
            g = idx - 1
            for (c, lo, hi) in srcs[g]:
                off = to_off[(idx, c)]
                nc.tensor.matmul(theta[:, hs],
                                 wtoS[:, off * 2 * D:(off + 1) * 2 * D],
                                 a3[c][:, hs],
                                 start=False, stop=True,
                                 skip_group_check=True)

        def emit_thsb(ch, hs, idx):
            nc.vector.tensor_scalar_add(theta_sb[:, hs], theta[:, hs], 0.0)

        def emit_exp(ch, hs, idx):
            if ch == 0:
                fetch_z(idx + 3)
            nc.scalar.activation(es[0:1, hs], ex[0:1, hs],
                                 mybir.ActivationFunctionType.Exp)

        def emit_mul(ch, hs, idx):
            nc.vector.tensor_tensor(t2[0:1, hs], es[0:1, hs],
                                    zrow[idx][0:1, hs], mybir.AluOpType.mult)

        def emit_add(ch, hs, idx):
            nc.vector.tensor_tensor(xi[idx % 2][0:1, hs], t2[0:1, hs],
                                    ex[D:D + 1, hs], mybir.AluOpType.add)
            # lazy xT row fill for later catch-ups + final output (SP idle)
            nc.sync.dma_start(xT[idx:idx + 1, hs], xi[idx % 2][0:1, hs])

        def emit_builds(idx):
            for fn in bsched.get(idx, []):
                fn()

        for ch, hs in chs:
            for idx in range(STOP):
                if ch == 0:
                    ph(ch, mk(emit_builds, idx))
                else:
                    ph(ch, lambda: None)
                if idx >= 1:
                    ph(ch, mk(emit_catchup, ch, hs, idx))
                    ph(ch, mk(emit_k1, ch, hs, idx))
                    ph(ch, mk(emit_relu, ch, hs, idx, 1))
                    ph(ch, mk(emit_grp, ch, hs, idx, 2))
                    ph(ch, mk(emit_hid_catchup, ch, hs, idx, 2))
                    ph(ch, mk(emit_relu, ch, hs, idx, 2))
                    ph(ch, mk(emit_grp, ch, hs, idx, 3))
                    ph(ch, mk(emit_hid_catchup, ch, hs, idx, 3))
                    ph(ch, mk(emit_relu, ch, hs, idx, 3))
                    ph(ch, mk(emit_extract, ch, hs, idx))
                    ph(ch, mk(emit_touch, ch, hs, idx))
                    ph(ch, mk(emit_thsb, ch, hs, idx))
                else:
                    for _ in range(9):
                        ph(ch, lambda: None)
                    ph(ch, mk(emit_extract, ch, hs, idx))
                    for _ in range(2):
                        ph(ch, lambda: None)
                ph(ch, mk(emit_exp, ch, hs, idx))
                ph(ch, mk(emit_mul, ch, hs, idx))
                ph(ch, mk(emit_add, ch, hs, idx))

        # interleaved emission with skew
        total = len(streams[0])
        for i in range(total + SKEW * NCH):
            for ch in range(NCH):
                j = i - ch * SKEW
                if 0 <= j < total:
                    streams[ch][j]()

        nc.scalar.activation(outf[:, :], xT[0:D, :],
                             mybir.ActivationFunctionType.Copy)
        nc.sync.dma_start(d_out, outf[:, :])

    nc.compile()
    return nc


_CACHE = {}


def _get_program(prep):
    if "nc" not in _CACHE:
        _CACHE["nc"] = _build(prep)
    return _CACHE["nc"]


def _in_maps(inputs, prep):
    import ml_dtypes
    bf16 = ml_dtypes.bfloat16
    z = np.asarray(inputs["z"], dtype=np.float32)
    maps = []
    for c in range(NCORES):
        zs = z[c * BC:(c + 1) * BC, :]                 # [512, 32]
        maps.append({
            "w0b": prep["w0b"].astype(bf16),
            "whu2": prep["whu2"].astype(bf16),
            "whu3": prep["whu3"].astype(bf16),
            "wou": prep["wou"].astype(bf16),
            "mcol": prep["mcol"],
            "w0k1": prep["w0k1"].astype(bf16),
            "ohx": prep["ohx"].astype(bf16),
            "bh0r": prep["bh0r"].astype(bf16),
            "bh1r": prep["bh1r"].astype(bf16),
            "boutr": prep["boutr"].astype(bf16),
            "zb": np.ascontiguousarray(zs.T),          # [32, 512] f32
        })
    return maps


def _prep_from_inputs(inputs):
    return _host_prep(np.asarray(inputs["W0"], np.float32),
                      np.asarray(inputs["b0"], np.float32),
                      np.asarray(inputs["Wh"], np.float32),
                      np.asarray(inputs["bh"], np.float32),
                      np.asarray(inputs["Wout"], np.float32),
                      np.asarray(inputs["bout"], np.float32))


def _run(inputs, trace=False):
    prep = _prep_from_inputs(inputs)
    nc = _get_program(prep)
    maps = _in_maps(inputs, prep)
    res = run_bass_kernel_spmd(nc, maps, core_ids=list(range(NCORES)),
                               trace=trace)
    out = np.empty((B, D), dtype=np.float32)
    for c in range(NCORES):
        out[c * BC:(c + 1) * BC, :] = res.results[c]["out"].T
    return out, res


def kernel(**inputs):
    out, _ = _run(inputs, trace=False)
    return out


# revision 49
# speedup vs baseline: 1.2740x; 1.2740x over previous
"""Incremental MADE autoregressive sampler on 8 TRN2 NeuronCores.

v2: ALL layer accumulators are persistent PSUM banks updated incrementally.

With hidden units degree-sorted, activations are append-only across the 32
autoregressive steps: once x_0..x_g are set, every hidden unit of degree <= g
is final. Exploit this at every layer:

- pre1 (L1 pre-activations): ping-pong PSUM banks per 128-block; one K=1
  rank-1 update per step (new coordinate), plus a single K=33 catch-up matmul
  from xT (with a ones-row folding in the bias) when a block first becomes
  the active cover.
- S2/S3 (L2/L3 pre-activations): ping-pong PSUM banks per cover block. Each
  step adds ONLY the newly finalized ~33-unit degree group (K~33 matmul).
  When a block is about to become cover, a catch-up chain (bias + full
  finalized chunks) accumulates the older prefix once. No per-step prefix
  recompute -> Tensor queue no longer head-of-line-blocks the serial chain.
- theta [64, B]: single persistent PSUM accumulator; per-step "touch" adds
  the new group's contribution to all output rows (K~33, M=64). The tail
  reads rows idx (mu) and 32+idx (log_std) straight out of PSUM.
- Tail: es=exp(theta_ls) (ACT) -> t2=es*z -> x_idx=t2+theta_mu written
  DIRECTLY into the xT row in SBUF (no DMA scatter; k1/catch-up read xT).

Batch: data-parallel over 8 cores (512 rows/core); each core runs two
256-wide column chains, phase-interleaved with a skew so the two serial
dependency chains overlap on different engines. Relus/tails are spread
across ACT/DVE/Pool(gpsimd) so no single elementwise engine saturates.
"""

import os
import sys
import numpy as np

for _p in ("/opt/trn_rl_repo", "/opt/pypackages"):
    if _p not in sys.path:
        sys.path.insert(0, _p)

import concourse.bass as bass
import concourse.tile as tile
from concourse import bacc
from concourse import mybir
from concourse.bass_utils import run_bass_kernel_spmd

D, H, L, B = 32, 1024, 2, 4096
NCORES = 8
BC = B // NCORES          # 512 batch rows per core
P = 128                   # partitions
NB = H // P               # 8 hidden blocks
F32 = mybir.dt.float32
MMDT = mybir.dt.bfloat16

STOP = int(os.environ.get("MADE_STOP", "32"))
NCH = int(os.environ.get("MADE_CHAINS", "2"))
SKEW = int(os.environ.get("MADE_SKEW", "7"))


def _schedule():
    """Static per-step schedule from the degree structure."""
    d_hid = np.arange(H) % (D - 1)
    perm = np.argsort(d_hid, kind="stable")
    ds = d_hid[perm]
    glo = [int(np.sum(ds < g)) for g in range(D - 1)]
    ghi = [int(np.sum(ds <= g)) for g in range(D - 1)]
    cov = {g: list(range(glo[g] // P, (ghi[g] - 1) // P + 1))
           for g in range(D - 1)}
    # src_slices(g): (block c, row lo, row hi) covering units [glo, ghi)
    srcs = {}
    for g in range(D - 1):
        sl = []
        for c in cov[g]:
            lo = max(glo[g], c * P) - c * P
            hi = min(ghi[g], (c + 1) * P) - c * P
            sl.append((c, lo, hi))
        srcs[g] = sl
    # first step idx at which block Bb is in cover
    first = {}
    for g in range(D - 1):
        for Bb in cov[g]:
            first.setdefault(Bb, g + 1)
    return perm, ds, glo, ghi, cov, srcs, first


def _host_prep(W0, b0, Wh, bh, Wout, bout):
    perm, ds, glo, ghi, cov, srcs, first = _schedule()
    d_in = np.arange(D)
    d_out = np.arange(D) - 1
    m0 = (ds[:, None] >= d_in[None, :]).astype(np.float32)        # [H, D]
    mh = (ds[:, None] >= ds[None, :]).astype(np.float32)          # [H, H]
    mo = (d_out[:, None] >= ds[None, :]).astype(np.float32)       # [D, H]
    mo = np.concatenate([mo, mo], axis=0)                         # [2D, H]

    W0p = m0 * W0[perm, :]                    # [H, D] (out, in)
    Wh0p = mh * Wh[0][perm][:, perm]          # [H, H] (out, in)
    Wh1p = mh * Wh[1][perm][:, perm]
    Wop = mo * Wout[:, perm]                  # [2D, H]

    w0b = np.concatenate([W0p.T, b0[perm][None, :]], axis=0)      # [D+1, H]
    wh0T = Wh0p.T.reshape(NB, P, H).copy()                        # [c][128, H]
    wh1T = Wh1p.T.reshape(NB, P, H).copy()

    # k1 packed columns: per (idx, Bb in cov(idx-1)): W0p[block Bb, idx-1]
    k1_off, k1_list = {}, []
    for idx in range(1, D):
        for Bb in cov[idx - 1]:
            k1_off[(idx, Bb)] = len(k1_list)
            k1_list.append(W0p[Bb * P:(Bb + 1) * P, idx - 1])
    w0k1 = np.concatenate(k1_list).reshape(1, -1)                 # [1, n*128]

    # slice maps for the device-built masked lhsT tensors
    g2_off, n_g = {}, 0
    for idx in range(1, D):
        g = idx - 1
        for Bb in cov[g]:
            for (c, lo, hi) in srcs[g]:
                g2_off[(idx, Bb, c)] = n_g
                n_g += 1
    to_off = {}
    for idx in range(1, D):
        for (c, lo, hi) in srcs[idx - 1]:
            to_off[(idx, c)] = len(to_off)
    n_to = len(to_off)

    # distinct group-row masks: per (g, c): rows [lo, hi) of block c
    m_off, m_cols = {}, []
    for g in range(D - 1):
        for (c, lo, hi) in srcs[g]:
            col = np.zeros(P, dtype=np.float32)
            col[lo:hi] = 1.0
            m_off[(g, c)] = len(m_cols)
            m_cols.append(col)
    mcol = np.stack(m_cols, axis=1)                               # [128, nm]

    # S2/S3 catch-up schedule (mirrors _build's hc_sched pair usage):
    # used (c src, Bb dst) weight blocks = catch-up pairs + group pairs
    pair_use = {}

    def use(c, Bb, step):
        k = (c, Bb)
        pair_use[k] = min(pair_use.get(k, 99), step)

    for Bb in range(1, NB):
        f = first[Bb]
        U = ghi[f - 2]
        cfull, rem = U // P, U % P
        for c in range(cfull + (1 if rem else 0)):
            use(c, Bb, max(1, f - 3))
    for idx in range(1, D):
        g = idx - 1
        for Bb in cov[g]:
            for (c, lo, hi) in srcs[g]:
                use(c, Bb, max(1, idx - 3))

    pairs = sorted(pair_use, key=lambda k: (pair_use[k], k))
    ub = {k: j for j, k in enumerate(pairs)}
    whu2 = np.concatenate([wh0T[c][:, Bb * P:(Bb + 1) * P]
                           for (c, Bb) in pairs], axis=1)         # [128, n*128]
    whu3 = np.concatenate([wh1T[c][:, Bb * P:(Bb + 1) * P]
                           for (c, Bb) in pairs], axis=1)

    # full output weights (small): wou[:, 64c:64(c+1)] = Wop[:, block c].T
    wou = np.concatenate([Wop[:, c * P:(c + 1) * P].T
                          for c in range(NB)], axis=1)            # [128, NB*2D]

    # one-hot theta_sb selectors, M=33 per idx: col 0 = e_{D+idx}, col 32 = e_idx
    M3 = D + 1
    ohx = np.zeros((2 * D, D * M3), dtype=np.float32)
    for idx in range(D):
        ohx[D + idx, M3 * idx] = 1.0
        ohx[idx, M3 * idx + D] = 1.0

    return dict(w0b=w0b, w0k1=w0k1, whu2=whu2, whu3=whu3, wou=wou,
                mcol=mcol, ohx=ohx,
                k1_off=k1_off, g2_off=g2_off, to_off=to_off, m_off=m_off,
                ub=ub, n_pairs=len(pairs), pair_use=pair_use,
                n_k1=len(k1_list), n_g2=n_g, n_to=n_to, n_m=len(m_cols),
                bh0r=bh[0][perm][None, :], bh1r=bh[1][perm][None, :],
                boutr=bout[None, :],
                glo=glo, ghi=ghi, cov=cov, srcs=srcs, first=first, ds=ds)


def _build(prep):
    nc = bacc.Bacc("TRN2", target_bir_lowering=False, debug=False,
                   num_devices=NCORES)

    def din(name, shape, dt=MMDT):
        return nc.dram_tensor(name, list(shape), dt, kind="ExternalInput").ap()

    NPAIR = prep["n_pairs"]
    d_w0b = din("w0b", (D + 1, H))
    d_whu2 = din("whu2", (P, NPAIR * P))
    d_whu3 = din("whu3", (P, NPAIR * P))
    d_wou = din("wou", (P, NB * 2 * D))
    d_mcol = din("mcol", (P, prep["n_m"]), F32)
    d_w0k1 = din("w0k1", (1, prep["n_k1"] * P))
    d_ohx = din("ohx", (2 * D, D * (D + 1)))
    d_bh0 = din("bh0r", (1, H))
    d_bh1 = din("bh1r", (1, H))
    d_bo = din("boutr", (1, 2 * D))
    d_z = din("zb", (D, BC), F32)
    d_out = nc.dram_tensor("out", [D, BC], F32, kind="ExternalOutput").ap()

    cov, srcs, first = prep["cov"], prep["srcs"], prep["first"]
    ghi, dsl = prep["ghi"], prep["ds"]
    # pre1 catch-up for block Bb is emitted during step first[Bb]-1
    catch_at = {}
    for Bb, f in first.items():
        if Bb >= 1:
            catch_at.setdefault(f - 1, []).append(Bb)

    # S2/S3 catch-up terms, spread over steps f-3..f-1 by data availability
    # (chunk c of the prefix is final once its last unit's group is done).
    # Terms of one (lyr, Bb) accumulation group stay in order; start/stop
    # flags mark the PSUM group boundaries.
    hc_sched = {}
    for lyr in (2, 3):
        for Bb in range(1, NB):
            f = first[Bb]
            U = ghi[f - 2]
            cfull, rem = U // P, U % P
            terms = [("bias", Bb)]
            terms += [("chunk", Bb, c) for c in range(cfull)]
            if rem:
                terms.append(("part", Bb, cfull, rem))
            n = len(terms)
            for j, t in enumerate(terms):
                if t[0] == "bias":
                    e = f - 3
                elif t[0] == "chunk":
                    e = max(f - 3, int(dsl[(t[2] + 1) * P - 1]) + 1)
                else:
                    e = f - 1
                e = min(max(e, 1), f - 1)
                hc_sched.setdefault((lyr, e), []).append((t, j == 0, j == n - 1))

    from contextlib import ExitStack
    with tile.TileContext(nc) as tc, ExitStack() as ctx:
        cp = ctx.enter_context(tc.tile_pool(name="const", bufs=1))
        pp = ctx.enter_context(tc.tile_pool(name="pers", bufs=1, space="PSUM"))

        w0b = cp.tile([D + 1, H], MMDT, tag="w0b")
        whu2 = cp.tile([P, NPAIR * P], MMDT, tag="whu2")
        whu3 = cp.tile([P, NPAIR * P], MMDT, tag="whu3")
        wou = cp.tile([P, NB * 2 * D], MMDT, tag="wou")
        mcol = cp.tile([P, prep["n_m"]], F32, tag="mcol")
        w0k1 = cp.tile([1, prep["n_k1"] * P], MMDT, tag="w0k1")
        # device-built masked lhsT slices (GPSIMD fills these from whu/wou)
        wgs2 = cp.tile([P, prep["n_g2"] * P], MMDT, tag="wgs2")
        wgs3 = cp.tile([P, prep["n_g2"] * P], MMDT, tag="wgs3")
        wtoS = cp.tile([P, prep["n_to"] * 2 * D], MMDT, tag="wtoS")
        wpmS = cp.tile([P, prep["n_to"] * (D + 1)], MMDT, tag="wpmS")
        bh0r = cp.tile([1, H], MMDT, tag="bh0r")
        bh1r = cp.tile([1, H], MMDT, tag="bh1r")
        bor = cp.tile([1, 2 * D], MMDT, tag="bor")
        ohx = cp.tile([2 * D, D * (D + 1)], MMDT, tag="ohx")
        theta_sb = cp.tile([2 * D, BC], MMDT, tag="theta_sb")
        ones = cp.tile([1, BC], MMDT, tag="ones")
        xT = cp.tile([D + 1, BC], MMDT, tag="xT")
        a1 = [cp.tile([P, BC], MMDT, tag=f"a1_{r}", name=f"a1_{r}") for r in range(NB)]
        a2 = [cp.tile([P, BC], MMDT, tag=f"a2_{r}", name=f"a2_{r}") for r in range(NB)]
        a3 = [cp.tile([P, BC], MMDT, tag=f"a3_{r}", name=f"a3_{r}") for r in range(NB)]
        es = cp.tile([1, BC], F32, tag="es")
        t2 = cp.tile([1, BC], F32, tag="t2")
        xi = [cp.tile([1, BC], MMDT, tag=f"xi{p}", name=f"xi{p}")
              for p in range(2)]
        outf = cp.tile([D, BC], F32, tag="outf")

        # persistent PSUM: pre1/S2/S3 ping-pong banks + theta accumulator +
        # extract tile (ls at row 0, mu at row 32: both %32-aligned)
        pre1 = [pp.tile([P, BC], F32, tag=f"pre1_{s}", name=f"pre1_{s}")
                for s in range(2)]
        s2 = [pp.tile([P, BC], F32, tag=f"s2_{s}", name=f"s2_{s}")
              for s in range(2)]
        s3 = [pp.tile([P, BC], F32, tag=f"s3_{s}", name=f"s3_{s}")
              for s in range(2)]
        theta = pp.tile([2 * D, BC], F32, tag="theta")
        ex = pp.tile([D + 1, BC], F32, tag="ex")

        # input DMAs, ordered by first use; whu columns are first-use ordered
        # so quarter k arrives before the steps that need it.
        nc.sync.dma_start(bor[:], d_bo)
        nc.sync.dma_start(ohx[:], d_ohx)
        nc.sync.dma_start(w0b[:], d_w0b)
        nc.sync.dma_start(w0k1[:], d_w0k1)
        nc.sync.dma_start(mcol[:], d_mcol)
        nc.sync.dma_start(wou[:], d_wou)
        nc.sync.dma_start(bh0r[:], d_bh0)
        nc.sync.dma_start(bh1r[:], d_bh1)
        bnd = [0] + [NPAIR * P * k // 4 for k in (1, 2, 3)] + [NPAIR * P]
        for a, b in zip(bnd, bnd[1:]):
            nc.sync.dma_start(whu2[:, a:b], d_whu2[:, a:b])
            nc.sync.dma_start(whu3[:, a:b], d_whu3[:, a:b])

        zrow = {}

        def fetch_z(i):
            if i < STOP and i not in zrow:
                zr_t = cp.tile([1, BC], F32, tag="zrow", bufs=4, name=f"zr{i}")
                zrow[i] = zr_t
                nc.sync.dma_start(zr_t[:], d_z[i:i + 1, :])

        for i in range(3):
            fetch_z(i)

        nc.vector.memset(xT[:], 0.0)
        nc.vector.memset(xT[D:D + 1, :], 1.0)
        nc.vector.memset(ones[:], 1.0)

        # theta := bout broadcast (rank-1); block-0 accumulators: bias+coords
        nc.tensor.matmul(theta[:, :], bor[0:1, :], ones[0:1, :],
                         start=True, stop=True, skip_group_check=True)
        nc.vector.tensor_scalar_add(theta_sb[:, :], theta[:, :], 0.0)
        nc.tensor.matmul(pre1[0], w0b[:, 0:P], xT[:, :],
                         start=True, stop=True, skip_group_check=True)
        nc.tensor.matmul(s2[0], bh0r[0:1, 0:P], ones[0:1, :],
                         start=True, stop=True, skip_group_check=True)
        nc.tensor.matmul(s3[0], bh1r[0:1, 0:P], ones[0:1, :],
                         start=True, stop=True, skip_group_check=True)

        CWX = BC // NCH
        chs = [(ch, slice(ch * CWX, (ch + 1) * CWX)) for ch in range(NCH)]

        # engine spread: (chain, layer 1/2/3) -> relu engine.
        # GPSIMD/Pool cannot access PSUM, so PSUM-reading ops (relu/exp/add)
        # go to ACT+DVE; Pool gets the SBUF-only tail multiply.
        RELU_ENG = {(0, 1): "act", (0, 2): "dve", (0, 3): "act",
                    (1, 1): "dve", (1, 2): "act", (1, 3): "dve"}

        def relu_op(eng, out_ap, in_ap):
            if eng == "act":
                nc.scalar.activation(out_ap, in_ap,
                                     mybir.ActivationFunctionType.Relu)
            elif eng == "dve":
                nc.vector.tensor_scalar_max(out_ap, in_ap, 0.0)
            else:
                nc.gpsimd.tensor_scalar_max(out_ap, in_ap, 0.0)



        streams = [[] for _ in range(NCH)]

        def ph(ch, fn):
            streams[ch].append(fn)

        def mk(fn, *args):
            return lambda a=args: fn(*a)

        k1_off = prep["k1_off"]
        g2_off, to_off = prep["g2_off"], prep["to_off"]
        m_off, ub = prep["m_off"], prep["ub"]
        M3 = D + 1

        # ---- device-side masked-slice builds (alternating DVE/ACT; GPSIMD
        # has a ~1.3us fixed cost per op so it is useless for these) ----
        nc.gpsimd.memset(wpmS[:], 0.0)
        _bt = [0]

        def bmul(out_ap, in_ap, m):
            _bt[0] += 1
            if _bt[0] % 2:
                nc.vector.tensor_scalar_mul(out_ap, in_ap, mcol[:, m:m + 1])
            else:
                nc.scalar.activation(out_ap, in_ap,
                                     mybir.ActivationFunctionType.Copy,
                                     scale=mcol[:, m:m + 1])

        def build_grp_slice(idx, Bb, c):
            g = idx - 1
            off = g2_off[(idx, Bb, c)]
            m = m_off[(g, c)]
            src = ub[(c, Bb)]
            for whu, wgs in ((whu2, wgs2), (whu3, wgs3)):
                bmul(wgs[:, off * P:(off + 1) * P],
                     whu[:, src * P:(src + 1) * P], m)

        def build_out_slice(idx, c):
            g = idx - 1
            off = to_off[(idx, c)]
            m = m_off[(g, c)]
            bmul(wtoS[:, off * 2 * D:(off + 1) * 2 * D],
                 wou[:, c * 2 * D:(c + 1) * 2 * D], m)
            for k, oc in ((0, D + idx), (D, idx)):
                bmul(wpmS[:, off * M3 + k:off * M3 + k + 1],
                     wou[:, c * 2 * D + oc:c * 2 * D + oc + 1], m)

        bsched = {}
        for idx in range(1, STOP):
            g = idx - 1
            at = max(0, idx - 3)
            for Bb in cov[g]:
                for (c, lo, hi) in srcs[g]:
                    bsched.setdefault(at, []).append(
                        lambda a=(idx, Bb, c): build_grp_slice(*a))
            for (c, lo, hi) in srcs[g]:
                bsched.setdefault(at, []).append(
                    lambda a=(idx, c): build_out_slice(*a))

        for fn in bsched.get(0, []):
            fn()

        def emit_catchup(ch, hs, idx):
            for Bb in catch_at.get(idx, []):
                nc.tensor.matmul(pre1[Bb % 2][:, hs],
                                 w0b[:, Bb * P:(Bb + 1) * P], xT[:, hs],
                                 start=True, stop=True, skip_group_check=True)

        def emit_k1(ch, hs, idx):
            g = idx - 1
            for Bb in cov[g]:
                off = k1_off[(idx, Bb)]
                nc.tensor.matmul(pre1[Bb % 2][:, hs],
                                 w0k1[0:1, off * P:(off + 1) * P],
                                 xi[(idx - 1) % 2][0:1, hs],
                                 start=False, stop=True, skip_group_check=True)

        def emit_relu(ch, hs, idx, lyr):
            g = idx - 1
            src, dst = {1: (pre1, a1), 2: (s2, a2), 3: (s3, a3)}[lyr]
            for Bb in cov[g]:
                relu_op(RELU_ENG[(ch % 2, lyr)], dst[Bb][:, hs],
                        src[Bb % 2][:, hs])

        def emit_grp(ch, hs, idx, lyr):
            """Add the newly final group g to cover-block accumulators."""
            g = idx - 1
            wg, sb, a_in = {2: (wgs2, s2, a1), 3: (wgs3, s3, a2)}[lyr]
            for Bb in cov[g]:
                for (c, lo, hi) in srcs[g]:
                    off = g2_off[(idx, Bb, c)]
                    nc.tensor.matmul(sb[Bb % 2][:, hs],
                                     wg[:, off * P:(off + 1) * P],
                                     a_in[c][:, hs],
                                     start=False, stop=True,
                                     skip_group_check=True)

        def emit_hid_catchup(ch, hs, idx, lyr):
            """Accumulate bias + finalized prefix for soon-to-be-cover blocks."""
            whu, sb, a_in, bias = {2: (whu2, s2, a1, bh0r),
                                   3: (whu3, s3, a2, bh1r)}[lyr]
            for (t, is_start, is_stop) in hc_sched.get((lyr, idx), []):
                Bb = t[1]
                dst = sb[Bb % 2][:, hs]
                if t[0] == "bias":
                    lh, rh = bias[0:1, Bb * P:(Bb + 1) * P], ones[0:1, hs]
                elif t[0] == "chunk":
                    c = t[2]
                    j = ub[(c, Bb)]
                    lh, rh = whu[:, j * P:(j + 1) * P], a_in[c][:, hs]
                else:
                    c, rem = t[2], t[3]
                    j = ub[(c, Bb)]
                    lh, rh = (whu[0:rem, j * P:(j + 1) * P],
                              a_in[c][0:rem, hs])
                nc.tensor.matmul(dst, lh, rh, start=is_start, stop=is_stop,
                                 skip_group_check=True)

        def emit_extract(ch, hs, idx):
            # One M=33 chain: ls_idx -> ex row 0, mu_idx -> ex row 32 (both
            # %32-aligned). Term 0 selects the theta_sb state rows (groups
            # <= idx-2); wpm terms add the current group's correction.
            cor = srcs[idx - 1] if idx >= 1 else []
            seq = [(ohx[:, M3 * idx:M3 * (idx + 1)], theta_sb[:, hs])]
            for (c, lo, hi) in cor:
                off = to_off[(idx, c)]
                seq.append((wpmS[:, M3 * off:M3 * (off + 1)], a3[c][:, hs]))
            for j, (lh, rh) in enumerate(seq):
                nc.tensor.matmul(ex[:, hs], lh, rh,
                                 start=(j == 0), stop=(j == len(seq) - 1),
                                 skip_group_check=True)

        def emit_touch(ch, hs, idx):
            g = idx - 1
            for (c, lo, hi) in srcs[g]:
                off = to_off[(idx, c)]
                nc.tensor.matmul(theta[:, hs],
                                 wtoS[:, off * 2 * D:(off + 1) * 2 * D],
                                 a3[c][:, hs],
                                 start=False, stop=True,
                                 skip_group_check=True)

        def emit_thsb(ch, hs, idx):
            nc.vector.tensor_scalar_add(theta_sb[:, hs], theta[:, hs], 0.0)

        def emit_exp(ch, hs, idx):
            if ch == 0:
                fetch_z(idx + 3)
            nc.scalar.activation(es[0:1, hs], ex[0:1, hs],
                                 mybir.ActivationFunctionType.Exp)

        def emit_mul(ch, hs, idx):
            nc.vector.tensor_tensor(t2[0:1, hs], es[0:1, hs],
                                    zrow[idx][0:1, hs], mybir.AluOpType.mult)

        def emit_add(ch, hs, idx):
            nc.vector.tensor_tensor(xi[idx % 2][0:1, hs], t2[0:1, hs],
                                    ex[D:D + 1, hs], mybir.AluOpType.add)
            # lazy xT row fill for later catch-ups + final output (SP idle)
            nc.sync.dma_start(xT[idx:idx + 1, hs], xi[idx % 2][0:1, hs])

        def emit_builds(idx):
            for fn in bsched.get(idx, []):
                fn()

        for ch, hs in chs:
            for idx in range(STOP):
                if ch == 0:
                    ph(ch, mk(emit_builds, idx))
                else:
                    ph(ch, lambda: None)
                if idx >= 1:
                    ph(ch, mk(emit_catchup, ch, hs, idx))
                    ph(ch, mk(emit_k1, ch, hs, idx))
                    ph(ch, mk(emit_relu, ch, hs, idx, 1))
                    ph(ch, mk(emit_grp, ch, hs, idx, 2))
                    ph(ch, mk(emit_hid_catchup, ch, hs, idx, 2))
                    ph(ch, mk(emit_relu, ch, hs, idx, 2))
                    ph(ch, mk(emit_grp, ch, hs, idx, 3))
                    ph(ch, mk(emit_hid_catchup, ch, hs, idx, 3))
                    ph(ch, mk(emit_relu, ch, hs, idx, 3))
                    ph(ch, mk(emit_extract, ch, hs, idx))
                    ph(ch, mk(emit_touch, ch, hs, idx))
                    ph(ch, mk(emit_thsb, ch, hs, idx))
                else:
                    for _ in range(9):
                        ph(ch, lambda: None)
                    ph(ch, mk(emit_extract, ch, hs, idx))
                    for _ in range(2):
                        ph(ch, lambda: None)
                ph(ch, mk(emit_exp, ch, hs, idx))
                ph(ch, mk(emit_mul, ch, hs, idx))
                ph(ch, mk(emit_add, ch, hs, idx))

        # interleaved emission with skew
        total = len(streams[0])
        for i in range(total + SKEW * NCH):
            for ch in range(NCH):
                j = i - ch * SKEW
                if 0 <= j < total:
                    streams[ch][j]()

        nc.scalar.activation(outf[:, :], xT[0:D, :],
                             mybir.ActivationFunctionType.Copy)
        nc.sync.dma_start(d_out, outf[:, :])

    nc.compile()
    return nc


_CACHE = {}


def _get_program(prep):
    if "nc" not in _CACHE:
        _CACHE["nc"] = _build(prep)
    return _CACHE["nc"]


def _in_maps(inputs, prep):
    import ml_dtypes
    bf16 = ml_dtypes.bfloat16
    z = np.asarray(inputs["z"], dtype=np.float32)
    maps = []
    for c in range(NCORES):
        zs = z[c * BC:(c + 1) * BC, :]                 # [512, 32]
        maps.append({
            "w0b": prep["w0b"].astype(bf16),
            "whu2": prep["whu2"].astype(bf16),
            "whu3": prep["whu3"].astype(bf16),
            "wou": prep["wou"].astype(bf16),
            "mcol": prep["mcol"],
            "w0k1": prep["w0k1"].astype(bf16),
            "ohx": prep["ohx"].astype(bf16),
            "bh0r": prep["bh0r"].astype(bf16),
            "bh1r": prep["bh1r"].astype(bf16),
            "boutr": prep["boutr"].astype(bf16),
            "zb": np.ascontiguousarray(zs.T),          # [32, 512] f32
        })
    return maps


def _prep_from_inputs(inputs):
    return _host_prep(np.asarray(inputs["W0"], np.float32),
                      np.asarray(inputs["b0"], np.float32),
                      np.asarray(inputs["Wh"], np.float32),
                      np.asarray(inputs["bh"], np.float32),
                      np.asarray(inputs["Wout"], np.float32),
                      np.asarray(inputs["bout"], np.float32))


def _run(inputs, trace=False):
    prep = _prep_from_inputs(inputs)
    nc = _get_program(prep)
    maps = _in_maps(inputs, prep)
    res = run_bass_kernel_spmd(nc, maps, core_ids=list(range(NCORES)),
                               trace=trace)
    out = np.empty((B, D), dtype=np.float32)
    for c in range(NCORES):
        out[c * BC:(c + 1) * BC, :] = res.results[c]["out"].T
    return out, res


def kernel(**inputs):
    out, _ = _run(inputs, trace=False)
    return out


# revision 53
# speedup vs baseline: 1.2975x; 1.0184x over previous
"""Incremental MADE autoregressive sampler on 8 TRN2 NeuronCores.

v2: ALL layer accumulators are persistent PSUM banks updated incrementally.

With hidden units degree-sorted, activations are append-only across the 32
autoregressive steps: once x_0..x_g are set, every hidden unit of degree <= g
is final. Exploit this at every layer:

- pre1 (L1 pre-activations): ping-pong PSUM banks per 128-block; one K=1
  rank-1 update per step (new coordinate), plus a single K=33 catch-up matmul
  from xT (with a ones-row folding in the bias) when a block first becomes
  the active cover.
- S2/S3 (L2/L3 pre-activations): ping-pong PSUM banks per cover block. Each
  step adds ONLY the newly finalized ~33-unit degree group (K~33 matmul).
  When a block is about to become cover, a catch-up chain (bias + full
  finalized chunks) accumulates the older prefix once. No per-step prefix
  recompute -> Tensor queue no longer head-of-line-blocks the serial chain.
- theta [64, B]: single persistent PSUM accumulator; per-step "touch" adds
  the new group's contribution to all output rows (K~33, M=64). The tail
  reads rows idx (mu) and 32+idx (log_std) straight out of PSUM.
- Tail: es=exp(theta_ls) (ACT) -> t2=es*z -> x_idx=t2+theta_mu written
  DIRECTLY into the xT row in SBUF (no DMA scatter; k1/catch-up read xT).

Batch: data-parallel over 8 cores (512 rows/core); each core runs two
256-wide column chains, phase-interleaved with a skew so the two serial
dependency chains overlap on different engines. Relus/tails are spread
across ACT/DVE/Pool(gpsimd) so no single elementwise engine saturates.
"""

import os
import sys
import numpy as np

for _p in ("/opt/trn_rl_repo", "/opt/pypackages"):
    if _p not in sys.path:
        sys.path.insert(0, _p)

import concourse.bass as bass
import concourse.tile as tile
from concourse import bacc
from concourse import mybir
from concourse.bass_utils import run_bass_kernel_spmd

D, H, L, B = 32, 1024, 2, 4096
NCORES = 8
BC = B // NCORES          # 512 batch rows per core
P = 128                   # partitions
NB = H // P               # 8 hidden blocks
F32 = mybir.dt.float32
MMDT = mybir.dt.bfloat16

STOP = int(os.environ.get("MADE_STOP", "32"))
NCH = int(os.environ.get("MADE_CHAINS", "2"))
SKEW = int(os.environ.get("MADE_SKEW", "7"))


def _schedule():
    """Static per-step schedule from the degree structure."""
    d_hid = np.arange(H) % (D - 1)
    perm = np.argsort(d_hid, kind="stable")
    ds = d_hid[perm]
    glo = [int(np.sum(ds < g)) for g in range(D - 1)]
    ghi = [int(np.sum(ds <= g)) for g in range(D - 1)]
    cov = {g: list(range(glo[g] // P, (ghi[g] - 1) // P + 1))
           for g in range(D - 1)}
    # src_slices(g): (block c, row lo, row hi) covering units [glo, ghi)
    srcs = {}
    for g in range(D - 1):
        sl = []
        for c in cov[g]:
            lo = max(glo[g], c * P) - c * P
            hi = min(ghi[g], (c + 1) * P) - c * P
            sl.append((c, lo, hi))
        srcs[g] = sl
    # first step idx at which block Bb is in cover
    first = {}
    for g in range(D - 1):
        for Bb in cov[g]:
            first.setdefault(Bb, g + 1)
    return perm, ds, glo, ghi, cov, srcs, first


def _host_prep(W0, b0, Wh, bh, Wout, bout):
    perm, ds, glo, ghi, cov, srcs, first = _schedule()
    d_in = np.arange(D)
    d_out = np.arange(D) - 1
    m0 = (ds[:, None] >= d_in[None, :]).astype(np.float32)        # [H, D]
    mh = (ds[:, None] >= ds[None, :]).astype(np.float32)          # [H, H]
    mo = (d_out[:, None] >= ds[None, :]).astype(np.float32)       # [D, H]
    mo = np.concatenate([mo, mo], axis=0)                         # [2D, H]

    W0p = m0 * W0[perm, :]                    # [H, D] (out, in)
    Wh0p = mh * Wh[0][perm][:, perm]          # [H, H] (out, in)
    Wh1p = mh * Wh[1][perm][:, perm]
    Wop = mo * Wout[:, perm]                  # [2D, H]

    w0b = np.concatenate([W0p.T, b0[perm][None, :]], axis=0)      # [D+1, H]
    wh0T = Wh0p.T.reshape(NB, P, H).copy()                        # [c][128, H]
    wh1T = Wh1p.T.reshape(NB, P, H).copy()

    # k1 packed columns: per (idx, Bb in cov(idx-1)): W0p[block Bb, idx-1]
    k1_off, k1_list = {}, []
    for idx in range(1, D):
        for Bb in cov[idx - 1]:
            k1_off[(idx, Bb)] = len(k1_list)
            k1_list.append(W0p[Bb * P:(Bb + 1) * P, idx - 1])
    w0k1 = np.concatenate(k1_list).reshape(1, -1)                 # [1, n*128]

    # slice maps for the device-built masked lhsT tensors
    g2_off, n_g = {}, 0
    for idx in range(1, D):
        g = idx - 1
        for Bb in cov[g]:
            for (c, lo, hi) in srcs[g]:
                g2_off[(idx, Bb, c)] = n_g
                n_g += 1
    to_off = {}
    for idx in range(1, D):
        for (c, lo, hi) in srcs[idx - 1]:
            to_off[(idx, c)] = len(to_off)
    n_to = len(to_off)

    # distinct group-row masks: per (g, c): rows [lo, hi) of block c
    m_off, m_cols = {}, []
    for g in range(D - 1):
        for (c, lo, hi) in srcs[g]:
            col = np.zeros(P, dtype=np.float32)
            col[lo:hi] = 1.0
            m_off[(g, c)] = len(m_cols)
            m_cols.append(col)
    mcol = np.stack(m_cols, axis=1)                               # [128, nm]

    # S2/S3 catch-up schedule (mirrors _build's hc_sched pair usage):
    # used (c src, Bb dst) weight blocks = catch-up pairs + group pairs
    pair_use = {}

    def use(c, Bb, step):
        k = (c, Bb)
        pair_use[k] = min(pair_use.get(k, 99), step)

    for Bb in range(1, NB):
        f = first[Bb]
        U = ghi[f - 2]
        cfull, rem = U // P, U % P
        for c in range(cfull + (1 if rem else 0)):
            use(c, Bb, max(1, f - 3))
    for idx in range(1, D):
        g = idx - 1
        for Bb in cov[g]:
            for (c, lo, hi) in srcs[g]:
                use(c, Bb, max(1, idx - 3))

    pairs = sorted(pair_use, key=lambda k: (pair_use[k], k))
    ub = {k: j for j, k in enumerate(pairs)}
    whu2 = np.concatenate([wh0T[c][:, Bb * P:(Bb + 1) * P]
                           for (c, Bb) in pairs], axis=1)         # [128, n*128]
    whu3 = np.concatenate([wh1T[c][:, Bb * P:(Bb + 1) * P]
                           for (c, Bb) in pairs], axis=1)

    # full output weights (small): wou[:, 64c:64(c+1)] = Wop[:, block c].T
    wou = np.concatenate([Wop[:, c * P:(c + 1) * P].T
                          for c in range(NB)], axis=1)            # [128, NB*2D]

    # one-hot theta_sb selectors, M=33 per idx: col 0 = e_{D+idx}, col 32 = e_idx
    M3 = D + 1
    ohx = np.zeros((2 * D, D * M3), dtype=np.float32)
    for idx in range(D):
        ohx[D + idx, M3 * idx] = 1.0
        ohx[idx, M3 * idx + D] = 1.0

    return dict(w0b=w0b, w0k1=w0k1, whu2=whu2, whu3=whu3, wou=wou,
                mcol=mcol, ohx=ohx,
                k1_off=k1_off, g2_off=g2_off, to_off=to_off, m_off=m_off,
                ub=ub, n_pairs=len(pairs), pair_use=pair_use,
                n_k1=len(k1_list), n_g2=n_g, n_to=n_to, n_m=len(m_cols),
                bh0r=bh[0][perm][None, :], bh1r=bh[1][perm][None, :],
                boutr=bout[None, :],
                glo=glo, ghi=ghi, cov=cov, srcs=srcs, first=first, ds=ds)


def _build(prep):
    nc = bacc.Bacc("TRN2", target_bir_lowering=False, debug=False,
                   num_devices=NCORES)

    def din(name, shape, dt=MMDT):
        return nc.dram_tensor(name, list(shape), dt, kind="ExternalInput").ap()

    NPAIR = prep["n_pairs"]
    d_w0b = din("w0b", (D + 1, H))
    d_whu2 = din("whu2", (P, NPAIR * P))
    d_whu3 = din("whu3", (P, NPAIR * P))
    d_wou = din("wou", (P, NB * 2 * D))
    d_mcol = din("mcol", (P, prep["n_m"]), F32)
    d_w0k1 = din("w0k1", (1, prep["n_k1"] * P))
    d_ohx = din("ohx", (2 * D, D * (D + 1)))
    d_bh0 = din("bh0r", (1, H))
    d_bh1 = din("bh1r", (1, H))
    d_bo = din("boutr", (1, 2 * D))
    d_z = din("zb", (D, BC), F32)
    d_out = nc.dram_tensor("out", [D, BC], F32, kind="ExternalOutput").ap()

    cov, srcs, first = prep["cov"], prep["srcs"], prep["first"]
    ghi, dsl = prep["ghi"], prep["ds"]
    # pre1 catch-up for block Bb is emitted during step first[Bb]-1
    catch_at = {}
    for Bb, f in first.items():
        if Bb >= 1:
            catch_at.setdefault(f - 1, []).append(Bb)

    # S2/S3 catch-up terms, spread over steps f-3..f-1 by data availability
    # (chunk c of the prefix is final once its last unit's group is done).
    # Terms of one (lyr, Bb) accumulation group stay in order; start/stop
    # flags mark the PSUM group boundaries.
    hc_sched = {}
    for lyr in (2, 3):
        for Bb in range(1, NB):
            f = first[Bb]
            U = ghi[f - 2]
            cfull, rem = U // P, U % P
            terms = [("bias", Bb)]
            terms += [("chunk", Bb, c) for c in range(cfull)]
            if rem:
                terms.append(("part", Bb, cfull, rem))
            n = len(terms)
            for j, t in enumerate(terms):
                if t[0] == "bias":
                    e = f - 3
                elif t[0] == "chunk":
                    e = max(f - 3, int(dsl[(t[2] + 1) * P - 1]) + 1)
                else:
                    e = f - 1
                e = min(max(e, 1), f - 1)
                hc_sched.setdefault((lyr, e), []).append((t, j == 0, j == n - 1))

    from contextlib import ExitStack
    with tile.TileContext(nc) as tc, ExitStack() as ctx:
        cp = ctx.enter_context(tc.tile_pool(name="const", bufs=1))
        pp = ctx.enter_context(tc.tile_pool(name="pers", bufs=1, space="PSUM"))

        w0b = cp.tile([D + 1, H], MMDT, tag="w0b")
        whu2 = cp.tile([P, NPAIR * P], MMDT, tag="whu2")
        whu3 = cp.tile([P, NPAIR * P], MMDT, tag="whu3")
        wou = cp.tile([P, NB * 2 * D], MMDT, tag="wou")
        mcol = cp.tile([P, prep["n_m"]], F32, tag="mcol")
        w0k1 = cp.tile([1, prep["n_k1"] * P], MMDT, tag="w0k1")
        # device-built masked lhsT slices (GPSIMD fills these from whu/wou)
        wgs2 = cp.tile([P, prep["n_g2"] * P], MMDT, tag="wgs2")
        wgs3 = cp.tile([P, prep["n_g2"] * P], MMDT, tag="wgs3")
        wtoS = cp.tile([P, prep["n_to"] * 2 * D], MMDT, tag="wtoS")
        wpmS = cp.tile([P, prep["n_to"] * (D + 1)], MMDT, tag="wpmS")
        bh0r = cp.tile([1, H], MMDT, tag="bh0r")
        bh1r = cp.tile([1, H], MMDT, tag="bh1r")
        bor = cp.tile([1, 2 * D], MMDT, tag="bor")
        ohx = cp.tile([2 * D, D * (D + 1)], MMDT, tag="ohx")
        theta_sb = cp.tile([2 * D, BC], MMDT, tag="theta_sb")
        ones = cp.tile([1, BC], MMDT, tag="ones")
        xT = cp.tile([D + 1, BC], MMDT, tag="xT")
        a1 = [cp.tile([P, BC], MMDT, tag=f"a1_{r}", name=f"a1_{r}") for r in range(NB)]
        a2 = [cp.tile([P, BC], MMDT, tag=f"a2_{r}", name=f"a2_{r}") for r in range(NB)]
        a3 = [cp.tile([P, BC], MMDT, tag=f"a3_{r}", name=f"a3_{r}") for r in range(NB)]
        es = cp.tile([1, BC], F32, tag="es")
        t2 = cp.tile([1, BC], F32, tag="t2")
        xi = [cp.tile([1, BC], MMDT, tag=f"xi{p}", name=f"xi{p}")
              for p in range(2)]
        outf = cp.tile([D, BC], F32, tag="outf")

        # persistent PSUM: pre1/S2/S3 ping-pong banks + theta accumulator +
        # extract tile (ls at row 0, mu at row 32: both %32-aligned)
        pre1 = [pp.tile([P, BC], F32, tag=f"pre1_{s}", name=f"pre1_{s}")
                for s in range(2)]
        s2 = [pp.tile([P, BC], F32, tag=f"s2_{s}", name=f"s2_{s}")
              for s in range(2)]
        s3 = [pp.tile([P, BC], F32, tag=f"s3_{s}", name=f"s3_{s}")
              for s in range(2)]
        theta = pp.tile([2 * D, BC], F32, tag="theta")
        ex = pp.tile([D + 1, BC], F32, tag="ex")

        # input DMAs, ordered by first use; whu columns are first-use ordered
        # so quarter k arrives before the steps that need it.
        nc.sync.dma_start(bor[:], d_bo)
        nc.sync.dma_start(ohx[:], d_ohx)
        nc.sync.dma_start(w0b[:], d_w0b)
        nc.sync.dma_start(w0k1[:], d_w0k1)
        nc.sync.dma_start(mcol[:], d_mcol)
        nc.sync.dma_start(wou[:], d_wou)
        nc.sync.dma_start(bh0r[:], d_bh0)
        nc.sync.dma_start(bh1r[:], d_bh1)
        # whu chunks sized so the first-needed pairs arrive first and small
        npair = NPAIR
        cuts = [0, min(4, npair), min(10, npair), min(18, npair),
                min(28, npair), npair]
        for a, b in zip(cuts, cuts[1:]):
            if b > a:
                nc.sync.dma_start(whu2[:, a * P:b * P], d_whu2[:, a * P:b * P])
                nc.sync.dma_start(whu3[:, a * P:b * P], d_whu3[:, a * P:b * P])

        zrow = {}

        def fetch_z(i):
            if i < STOP and i not in zrow:
                zr_t = cp.tile([1, BC], F32, tag="zrow", bufs=4, name=f"zr{i}")
                zrow[i] = zr_t
                nc.sync.dma_start(zr_t[:], d_z[i:i + 1, :])

        for i in range(3):
            fetch_z(i)

        nc.vector.memset(xT[:], 0.0)
        nc.vector.memset(xT[D:D + 1, :], 1.0)
        nc.vector.memset(ones[:], 1.0)

        # theta := bout broadcast (rank-1); block-0 accumulators: bias+coords
        nc.tensor.matmul(theta[:, :], bor[0:1, :], ones[0:1, :],
                         start=True, stop=True, skip_group_check=True)
        nc.vector.tensor_scalar_add(theta_sb[:, :], theta[:, :], 0.0)
        nc.tensor.matmul(pre1[0], w0b[:, 0:P], xT[:, :],
                         start=True, stop=True, skip_group_check=True)
        nc.tensor.matmul(s2[0], bh0r[0:1, 0:P], ones[0:1, :],
                         start=True, stop=True, skip_group_check=True)
        nc.tensor.matmul(s3[0], bh1r[0:1, 0:P], ones[0:1, :],
                         start=True, stop=True, skip_group_check=True)

        CWX = BC // NCH
        chs = [(ch, slice(ch * CWX, (ch + 1) * CWX)) for ch in range(NCH)]

        # engine spread: (chain, layer 1/2/3) -> relu engine.
        # GPSIMD/Pool cannot access PSUM, so PSUM-reading ops (relu/exp/add)
        # go to ACT+DVE; Pool gets the SBUF-only tail multiply.
        RELU_ENG = {(0, 1): "act", (0, 2): "dve", (0, 3): "act",
                    (1, 1): "dve", (1, 2): "act", (1, 3): "dve"}

        def relu_op(eng, out_ap, in_ap):
            if eng == "act":
                nc.scalar.activation(out_ap, in_ap,
                                     mybir.ActivationFunctionType.Relu)
            elif eng == "dve":
                nc.vector.tensor_scalar_max(out_ap, in_ap, 0.0)
            else:
                nc.gpsimd.tensor_scalar_max(out_ap, in_ap, 0.0)



        streams = [[] for _ in range(NCH)]

        def ph(ch, fn):
            streams[ch].append(fn)

        def mk(fn, *args):
            return lambda a=args: fn(*a)

        k1_off = prep["k1_off"]
        g2_off, to_off = prep["g2_off"], prep["to_off"]
        m_off, ub = prep["m_off"], prep["ub"]
        M3 = D + 1

        # ---- device-side masked-slice builds (alternating DVE/ACT; GPSIMD
        # has a ~1.3us fixed cost per op so it is useless for these) ----
        nc.gpsimd.memset(wpmS[:], 0.0)
        _bt = [0]

        def bmul(out_ap, in_ap, m):
            _bt[0] += 1
            if _bt[0] % 2:
                nc.vector.tensor_scalar_mul(out_ap, in_ap, mcol[:, m:m + 1])
            else:
                nc.scalar.activation(out_ap, in_ap,
                                     mybir.ActivationFunctionType.Copy,
                                     scale=mcol[:, m:m + 1])

        def build_grp_slice(idx, Bb, c):
            g = idx - 1
            off = g2_off[(idx, Bb, c)]
            m = m_off[(g, c)]
            src = ub[(c, Bb)]
            for whu, wgs in ((whu2, wgs2), (whu3, wgs3)):
                bmul(wgs[:, off * P:(off + 1) * P],
                     whu[:, src * P:(src + 1) * P], m)

        def build_out_slice(idx, c):
            g = idx - 1
            off = to_off[(idx, c)]
            m = m_off[(g, c)]
            bmul(wtoS[:, off * 2 * D:(off + 1) * 2 * D],
                 wou[:, c * 2 * D:(c + 1) * 2 * D], m)
            for k, oc in ((0, D + idx), (D, idx)):
                bmul(wpmS[:, off * M3 + k:off * M3 + k + 1],
                     wou[:, c * 2 * D + oc:c * 2 * D + oc + 1], m)

        bsched = {}
        for idx in range(1, STOP):
            g = idx - 1
            at = max(0, idx - 3)
            for Bb in cov[g]:
                for (c, lo, hi) in srcs[g]:
                    bsched.setdefault(at, []).append(
                        lambda a=(idx, Bb, c): build_grp_slice(*a))
            for (c, lo, hi) in srcs[g]:
                bsched.setdefault(at, []).append(
                    lambda a=(idx, c): build_out_slice(*a))

        def emit_catchup(ch, hs, idx):
            for Bb in catch_at.get(idx, []):
                nc.tensor.matmul(pre1[Bb % 2][:, hs],
                                 w0b[:, Bb * P:(Bb + 1) * P], xT[:, hs],
                                 start=True, stop=True, skip_group_check=True)

        def emit_k1(ch, hs, idx):
            g = idx - 1
            for Bb in cov[g]:
                off = k1_off[(idx, Bb)]
                nc.tensor.matmul(pre1[Bb % 2][:, hs],
                                 w0k1[0:1, off * P:(off + 1) * P],
                                 xi[(idx - 1) % 2][0:1, hs],
                                 start=False, stop=True, skip_group_check=True)

        def emit_relu(ch, hs, idx, lyr):
            g = idx - 1
            src, dst = {1: (pre1, a1), 2: (s2, a2), 3: (s3, a3)}[lyr]
            for Bb in cov[g]:
                relu_op(RELU_ENG[(ch % 2, lyr)], dst[Bb][:, hs],
                        src[Bb % 2][:, hs])

        def emit_grp(ch, hs, idx, lyr):
            """Add the newly final group g to cover-block accumulators."""
            g = idx - 1
            wg, sb, a_in = {2: (wgs2, s2, a1), 3: (wgs3, s3, a2)}[lyr]
            for Bb in cov[g]:
                for (c, lo, hi) in srcs[g]:
                    off = g2_off[(idx, Bb, c)]
                    nc.tensor.matmul(sb[Bb % 2][:, hs],
                                     wg[:, off * P:(off + 1) * P],
                                     a_in[c][:, hs],
                                     start=False, stop=True,
                                     skip_group_check=True)

        def emit_hid_catchup(ch, hs, idx, lyr):
            """Accumulate bias + finalized prefix for soon-to-be-cover blocks."""
            whu, sb, a_in, bias = {2: (whu2, s2, a1, bh0r),
                                   3: (whu3, s3, a2, bh1r)}[lyr]
            for (t, is_start, is_stop) in hc_sched.get((lyr, idx), []):
                Bb = t[1]
                dst = sb[Bb % 2][:, hs]
                if t[0] == "bias":
                    lh, rh = bias[0:1, Bb * P:(Bb + 1) * P], ones[0:1, hs]
                elif t[0] == "chunk":
                    c = t[2]
                    j = ub[(c, Bb)]
                    lh, rh = whu[:, j * P:(j + 1) * P], a_in[c][:, hs]
                else:
                    c, rem = t[2], t[3]
                    j = ub[(c, Bb)]
                    lh, rh = (whu[0:rem, j * P:(j + 1) * P],
                              a_in[c][0:rem, hs])
                nc.tensor.matmul(dst, lh, rh, start=is_start, stop=is_stop,
                                 skip_group_check=True)

        def emit_extract(ch, hs, idx):
            # One M=33 chain: ls_idx -> ex row 0, mu_idx -> ex row 32 (both
            # %32-aligned). Term 0 selects the theta_sb state rows (groups
            # <= idx-2); wpm terms add the current group's correction.
            cor = srcs[idx - 1] if idx >= 1 else []
            seq = [(ohx[:, M3 * idx:M3 * (idx + 1)], theta_sb[:, hs])]
            for (c, lo, hi) in cor:
                off = to_off[(idx, c)]
                seq.append((wpmS[:, M3 * off:M3 * (off + 1)], a3[c][:, hs]))
            for j, (lh, rh) in enumerate(seq):
                nc.tensor.matmul(ex[:, hs], lh, rh,
                                 start=(j == 0), stop=(j == len(seq) - 1),
                                 skip_group_check=True)

        def emit_touch(ch, hs, idx):
            g = idx - 1
            for (c, lo, hi) in srcs[g]:
                off = to_off[(idx, c)]
                nc.tensor.matmul(theta[:, hs],
                                 wtoS[:, off * 2 * D:(off + 1) * 2 * D],
                                 a3[c][:, hs],
                                 start=False, stop=True,
                                 skip_group_check=True)

        def emit_thsb(ch, hs, idx):
            nc.vector.tensor_scalar_add(theta_sb[:, hs], theta[:, hs], 0.0)

        def emit_exp(ch, hs, idx):
            if ch == 0:
                fetch_z(idx + 3)
            nc.scalar.activation(es[0:1, hs], ex[0:1, hs],
                                 mybir.ActivationFunctionType.Exp)

        def emit_mul(ch, hs, idx):
            nc.vector.tensor_tensor(t2[0:1, hs], es[0:1, hs],
                                    zrow[idx][0:1, hs], mybir.AluOpType.mult)

        def emit_add(ch, hs, idx):
            nc.vector.tensor_tensor(xi[idx % 2][0:1, hs], t2[0:1, hs],
                                    ex[D:D + 1, hs], mybir.AluOpType.add)
            # lazy xT row fill for later catch-ups + final output (SP idle)
            nc.sync.dma_start(xT[idx:idx + 1, hs], xi[idx % 2][0:1, hs])

        def emit_builds(idx):
            for fn in bsched.get(idx, []):
                fn()

        for ch, hs in chs:
            for idx in range(STOP):
                if idx >= 1:
                    ph(ch, mk(emit_catchup, ch, hs, idx))
                    ph(ch, mk(emit_k1, ch, hs, idx))
                    ph(ch, mk(emit_relu, ch, hs, idx, 1))
                    ph(ch, mk(emit_grp, ch, hs, idx, 2))
                    ph(ch, mk(emit_hid_catchup, ch, hs, idx, 2))
                    ph(ch, mk(emit_relu, ch, hs, idx, 2))
                    ph(ch, mk(emit_grp, ch, hs, idx, 3))
                    ph(ch, mk(emit_hid_catchup, ch, hs, idx, 3))
                    ph(ch, mk(emit_relu, ch, hs, idx, 3))
                    ph(ch, mk(emit_extract, ch, hs, idx))
                    ph(ch, mk(emit_touch, ch, hs, idx))
                    ph(ch, mk(emit_thsb, ch, hs, idx))
                else:
                    for _ in range(9):
                        ph(ch, lambda: None)
                    ph(ch, mk(emit_extract, ch, hs, idx))
                    for _ in range(2):
                        ph(ch, lambda: None)
                ph(ch, mk(emit_exp, ch, hs, idx))
                ph(ch, mk(emit_mul, ch, hs, idx))
                ph(ch, mk(emit_add, ch, hs, idx))
                if ch == 0:
                    ph(ch, mk(emit_builds, idx))
                else:
                    ph(ch, lambda: None)

        # interleaved emission with skew
        total = len(streams[0])
        for i in range(total + SKEW * NCH):
            for ch in range(NCH):
                j = i - ch * SKEW
                if 0 <= j < total:
                    streams[ch][j]()

        nc.scalar.activation(outf[:, :], xT[0:D, :],
                             mybir.ActivationFunctionType.Copy)
        nc.sync.dma_start(d_out, outf[:, :])

    nc.compile()
    return nc


_CACHE = {}


def _get_program(prep):
    if "nc" not in _CACHE:
        _CACHE["nc"] = _build(prep)
    return _CACHE["nc"]


def _in_maps(inputs, prep):
    import ml_dtypes
    bf16 = ml_dtypes.bfloat16
    z = np.asarray(inputs["z"], dtype=np.float32)
    maps = []
    for c in range(NCORES):
        zs = z[c * BC:(c + 1) * BC, :]                 # [512, 32]
        maps.append({
            "w0b": prep["w0b"].astype(bf16),
            "whu2": prep["whu2"].astype(bf16),
            "whu3": prep["whu3"].astype(bf16),
            "wou": prep["wou"].astype(bf16),
            "mcol": prep["mcol"],
            "w0k1": prep["w0k1"].astype(bf16),
            "ohx": prep["ohx"].astype(bf16),
            "bh0r": prep["bh0r"].astype(bf16),
            "bh1r": prep["bh1r"].astype(bf16),
            "boutr": prep["boutr"].astype(bf16),
            "zb": np.ascontiguousarray(zs.T),          # [32, 512] f32
        })
    return maps


def _prep_from_inputs(inputs):
    return _host_prep(np.asarray(inputs["W0"], np.float32),
                      np.asarray(inputs["b0"], np.float32),
                      np.asarray(inputs["Wh"], np.float32),
                      np.asarray(inputs["bh"], np.float32),
                      np.asarray(inputs["Wout"], np.float32),
                      np.asarray(inputs["bout"], np.float32))


def _run(inputs, trace=False):
    prep = _prep_from_inputs(inputs)
    nc = _get_program(prep)
    maps = _in_maps(inputs, prep)
    res = run_bass_kernel_spmd(nc, maps, core_ids=list(range(NCORES)),
                               trace=trace)
    out = np.empty((B, D), dtype=np.float32)
    for c in range(NCORES):
        out[c * BC:(c + 1) * BC, :] = res.results[c]["out"].T
    return out, res


def kernel(**inputs):
    out, _ = _run(inputs, trace=False)
    return out
